# revision 30
# baseline (speedup 1.0000x reference)
"""Trainium2 Bass kernel for nn_LocationAwareMSAGAT_Net.

Strategy: data-parallel over batch B=8 across the 8 NeuronCores (one batch
element per core); all parameters replicated.  Per core:

  phase A: multi-scale dilated conv (as 24 shifted matmuls, bf16) + BN fold
           + SiLU (ScalarE, conv bias folded into activation bias)
  phase B: bottleneck (alpha folded into W_low; accumulated in PSUM over
           scales) -> W_high -> +residual -> LayerNorm1 -> transpose (PE)
  phase C: GAT projections: one matmul computes Wh for all heads plus
           src/dst attention logits (gat_W@a_src / gat_W@a_dst appended as
           extra columns)
  phase D: attention, computed transposed (P^T[m,q] tiles), per head:
           srcb = broadcast src (replicated-column matmul, PE)
           s1   = srcb + dst         (DVE tensor_scalar, per-partition dst)
           s1  += maskNEG            (DVE tensor_tensor, in halves)
           leaky-relu + exp, two flavors to balance engines:
             V-path heads: t2 = 0.2*s1 (DVE 4x), ptl = max(s1,t2) (DVE 2x),
                           pt = exp(ptl) (ScalarE)
             S-path heads: ptl = Lrelu(s1, alpha=0.2) (ScalarE),
                           pt = exp(ptl) (ScalarE)
           hp^T = [Wh_h | ones]^T @ P^T accumulated in PSUM over m-chunks
           (ones column yields softmax denominators)
           PE-transpose back, divide rows by denominator
  phase E: LayerNorm2 -> DMA out

Everything on the PE is bf16 with fp32 PSUM accumulation.
"""

import os
import numpy as np
import ml_dtypes
from contextlib import ExitStack

import concourse.bass as bass
import concourse.tile as tile
from concourse import bacc, mybir
from concourse.bass_utils import run_bass_kernel_spmd
from concourse.masks import make_identity

BF = mybir.dt.bfloat16
F32 = mybir.dt.float32
EPS = 1e-5
NEG = -1e9

B, N, H = 8, 1024, 256
S, K, HEADS = 4, 3, 4
D = H // HEADS          # 64
NCH = N // 128          # 8 chunks of 128
CCH = H // 128          # 2 channel chunks
BOT = 8                 # bottleneck dim

V_PATH = (0, 2)         # heads whose leaky-relu runs on DVE; rest on ScalarE
HEAD_ORDER = (1, 0, 3, 2)  # S-path heads early so the tail head is V-path
S_HEADS = tuple(h for h in range(HEADS) if h not in V_PATH)

_CACHED = {}


def _build(trivial: dict) -> bass.Bass:
    nc = bacc.Bacc("TRN2", target_bir_lowering=False, debug=False,
                   num_devices=B)

    xt_d = nc.declare_dram_parameter("xt", [128, CCH, N], BF, isOutput=False)
    xres_d = nc.declare_dram_parameter("xres", [128, NCH, H], F32, isOutput=False)
    wt_d = nc.declare_dram_parameter("wt", [128, S * K * CCH, H], BF, isOutput=False)
    bconv_d = nc.declare_dram_parameter("bconv", [128, S * CCH], F32, isOutput=False)
    wlow_d = nc.declare_dram_parameter("wlow", [128, S * CCH, BOT], BF, isOutput=False)
    whigh_d = nc.declare_dram_parameter("whigh", [BOT, H], BF, isOutput=False)
    g_d = nc.declare_dram_parameter("gmat", [128, CCH, H + 2 * HEADS], BF,
                                    isOutput=False)
    mask_d = nc.declare_dram_parameter("maskT", [128, NCH, N], BF, isOutput=False)
    wsr_d = nc.declare_dram_parameter("wsrcrep", [128, HEADS, CCH, 128], BF,
                                      isOutput=False)
    sel_d = nc.declare_dram_parameter("csdsel", [33, 2, 2, 2], BF,
                                      isOutput=False)
    out_d = nc.declare_dram_parameter("out", [N, H], F32, isOutput=True)

    with tile.TileContext(nc) as tc:
        with ExitStack() as ctx:
            _body(ctx, tc, xt_d, xres_d, wt_d, bconv_d, wlow_d, whigh_d, g_d,
                  mask_d, wsr_d, sel_d, out_d)
    nc.compile()
    return nc


def _body(ctx, tc, xt_d, xres_d, wt_d, bconv_d, wlow_d, whigh_d, g_d,
          mask_d, wsr_d, sel_d, out_d):
    nc = tc.nc
    consts = ctx.enter_context(tc.tile_pool(name="consts", bufs=1))
    work = ctx.enter_context(tc.tile_pool(name="work", bufs=3))
    statp = ctx.enter_context(tc.tile_pool(name="stats", bufs=4))
    outp = ctx.enter_context(tc.tile_pool(name="outp", bufs=3))

    ctxA = ExitStack()
    apool = ctxA.enter_context(tc.tile_pool(name="apool", bufs=1))

    # ---------------- constants / inputs into SBUF ----------------
    # sync queue: phase-A-critical bytes, in consumption order.
    # gpsimd (software DGE) queue: everything needed later (mask last).
    xpad = apool.tile([128, CCH, N + 16], BF, tag="xpad")
    nc.vector.memset(xpad[:, :, 0:8], 0.0)
    nc.vector.memset(xpad[:, :, N + 8:N + 16], 0.0)
    for c in range(CCH):
        nc.sync.dma_start(out=xpad[:, c, 8:8 + N], in_=xt_d[:, c, :])

    bconv_sb = apool.tile([128, S * CCH], F32, tag="bconv")
    nc.sync.dma_start(out=bconv_sb[:], in_=bconv_d[:])

    wt_sbs = []
    for i in range(S):
        w = apool.tile([128, K * CCH, H], BF, tag=f"wt{i}")
        nc.sync.dma_start(out=w[:],
                          in_=wt_d[:, i * K * CCH:(i + 1) * K * CCH, :])
        wt_sbs.append(w)

    wlow_sb = apool.tile([128, S * CCH, BOT], BF, tag="wlow")
    nc.sync.dma_start(out=wlow_sb[:], in_=wlow_d[:])

    whigh_sb = consts.tile([BOT, H], BF, tag="whigh")
    nc.sync.dma_start(out=whigh_sb[:], in_=whigh_d[:])

    xres_sb = consts.tile([128, NCH, H], F32, tag="xres")
    nc.sync.dma_start(out=xres_sb[:], in_=xres_d[:])

    g_sb = consts.tile([128, CCH, H + 2 * HEADS], BF, tag="gmat")
    nc.sync.dma_start(out=g_sb[:], in_=g_d[:])

    wsr_sb = consts.tile([128, HEADS, CCH, 128], BF, tag="wsr")
    nc.sync.dma_start(out=wsr_sb[:], in_=wsr_d[:])

    mask_sb = consts.tile([128, NCH, N], BF, tag="mask")
    nc.sync.dma_start(out=mask_sb[:], in_=mask_d[:])

    ident_bf = consts.tile([128, 128], BF, tag="idbf")
    make_identity(nc, ident_bf[:])
    ident_f32 = consts.tile([128, 128], F32, tag="idf32")
    make_identity(nc, ident_f32[:])
    eps_sb = consts.tile([128, 1], F32, tag="eps")
    nc.vector.memset(eps_sb[:], EPS)
    zero_sb = consts.tile([128, 1], F32, tag="zero")
    nc.vector.memset(zero_sb[:], 0.0)

    # persistent intermediates
    fused_sb = apool.tile([128, S, CCH, N], BF, tag="fused")
    lowT_sb = consts.tile([BOT, N], BF, tag="lowT")
    h_all = consts.tile([128, NCH, H], F32, tag="h_all")
    mv1 = consts.tile([128, NCH, 2], F32, tag="mv1")
    rstd1 = consts.tile([128, NCH], F32, tag="rstd1")
    hT_sb = consts.tile([128, CCH, N], BF, tag="hT")
    wh_all = consts.tile([128, NCH, HEADS * (D + 1)], BF, tag="wh")
    nc.vector.memset(
        wh_all[:].rearrange("p j (h x) -> p j h x", x=D + 1)[:, :, :, D], 1.0)
    sd_sb = consts.tile([128, NCH, 2 * HEADS], F32, tag="sd")
    hp_all = consts.tile([128, NCH, H], F32, tag="hp")
    mv2 = consts.tile([128, NCH, 2], F32, tag="mv2")
    rstd2 = consts.tile([128, NCH], F32, tag="rstd2")

    sim_compat = os.environ.get("BASS_SIM_COMPAT", "0") == "1"

    # ---------------- phase A: conv + silu (nch-outer so phase B of each
    # N-half overlaps the other half's conv matmuls) ----------------
    ctxAB = ExitStack()
    psB = ctxAB.enter_context(tc.tile_pool(name="psB", bufs=2, space="PSUM"))
    convp = ctxA.enter_context(tc.tile_pool(name="convp", bufs=4, space="PSUM"))
    lowp = ctxA.enter_context(tc.tile_pool(name="lowp", bufs=2, space="PSUM"))
    for nch in range(2):
        for i in range(S):
            for cout in range(CCH):
                ps = convp.tile([128, 512], F32, tag="conv")
                dil = 2 ** i
                first = True
                for c in range(CCH):
                    for k in range(K):
                        sh = (k - 1) * dil
                        t = k * CCH + c
                        nc.tensor.matmul(
                            ps[:],
                            lhsT=wt_sbs[i][:, t, cout * 128:(cout + 1) * 128],
                            rhs=xpad[:, c, 8 + sh + nch * 512:
                                     8 + sh + nch * 512 + 512],
                            start=first, stop=(c == CCH - 1 and k == K - 1))
                        first = False
                dst = fused_sb[:, i, cout, nch * 512:nch * 512 + 512]
                bias_ap = bconv_sb[:, i * CCH + cout:i * CCH + cout + 1]
                if sim_compat:
                    # CoreSim has no Silu: sigmoid + fused (ps+b)*sig on DVE
                    sg = work.tile([128, 512], F32, tag="sg")
                    nc.scalar.activation(
                        out=sg[:], in_=ps[:],
                        func=mybir.ActivationFunctionType.Sigmoid,
                        bias=bias_ap, scale=1.0)
                    nc.vector.scalar_tensor_tensor(
                        out=dst, in0=ps[:], scalar=bias_ap, in1=sg[:],
                        op0=mybir.AluOpType.add, op1=mybir.AluOpType.mult)
                else:
                    nc.scalar.activation(
                        out=dst, in_=ps[:],
                        func=mybir.ActivationFunctionType.Silu,
                        bias=bias_ap, scale=1.0)

        # A2 for this half: lowT = sum_i (a_i W_low)^T @ silu_i
        lps = lowp.tile([BOT, 512], F32, tag="low")
        first = True
        for i in range(S):
            for c in range(CCH):
                nc.tensor.matmul(
                    lps[:],
                    lhsT=wlow_sb[:, i * CCH + c, :],
                    rhs=fused_sb[:, i, c, nch * 512:nch * 512 + 512],
                    start=first, stop=(i == S - 1 and c == CCH - 1))
                first = False
        nc.vector.tensor_copy(out=lowT_sb[:, nch * 512:nch * 512 + 512],
                              in_=lps[:])

        # B part 1 for this half: high + residual + stats
        for q in range(nch * 4, nch * 4 + 4):
            hps = psB.tile([128, H], F32, tag="high")
            nc.tensor.matmul(hps[:], lhsT=lowT_sb[:, q * 128:(q + 1) * 128],
                             rhs=whigh_sb[:], start=True, stop=True)
            nc.vector.tensor_add(h_all[:, q, :], hps[:], xres_sb[:, q, :])
            st = statp.tile([128, 6], F32, tag="bn1")
            nc.vector.bn_stats(out=st[:], in_=h_all[:, q, :])
            nc.vector.bn_aggr(out=mv1[:, q, :], in_=st[:])
    ctxA.close()
    ctxAB.close()

    # ---------------- phase B2: ln1 + transpose, pipelined into C ----------
    ctxB = ExitStack()
    psTrB = ctxB.enter_context(tc.tile_pool(name="psTrB", bufs=3, space="PSUM"))
    psC = ctxB.enter_context(tc.tile_pool(name="psC", bufs=2, space="PSUM"))
    # rstd1 = exp(-0.5 * ln(var + eps)), batched: Ln and Exp live in
    # different walrus table sets, so interleaving them thrashes
    # ACT_TABLE_LOAD (~1.3us each).  Two batched calls -> two loads total.
    nc.scalar.activation(out=rstd1[:], in_=mv1[:, :, 1],
                         func=mybir.ActivationFunctionType.Ln,
                         bias=eps_sb[:], scale=1.0)
    nc.scalar.activation(out=rstd1[:], in_=rstd1[:],
                         func=mybir.ActivationFunctionType.Exp,
                         bias=zero_sb[:], scale=-0.5)
    for q in range(NCH):
        hn = work.tile([128, H], BF, tag="hn")
        nc.vector.tensor_scalar(
            out=hn[:], in0=h_all[:, q, :],
            scalar1=mv1[:, q, 0:1], scalar2=rstd1[:, q:q + 1],
            op0=mybir.AluOpType.subtract, op1=mybir.AluOpType.mult)
        for c in range(CCH):
            tp = psTrB.tile([128, 128], BF, tag="trh")
            nc.tensor.transpose(out=tp[:],
                                in_=hn[:, c * 128:(c + 1) * 128],
                                identity=ident_bf[:])
            nc.scalar.copy(out=hT_sb[:, c, q * 128:(q + 1) * 128],
                           in_=tp[:])
        # ------- phase C for this chunk: GAT projections -------
        gps = psC.tile([128, H + 2 * HEADS], F32, tag="gat")
        for c in range(CCH):
            nc.tensor.matmul(gps[:], lhsT=hT_sb[:, c, q * 128:(q + 1) * 128],
                             rhs=g_sb[:, c, :], start=(c == 0),
                             stop=(c == CCH - 1))
        whj = wh_all[:, q, :].rearrange("p (h x) -> p h x", x=D + 1)
        nc.vector.tensor_copy(
            out=whj[:, :, 0:D],
            in_=gps[:, 0:H].rearrange("p (h x) -> p h x", x=D))
        nc.vector.tensor_copy(out=sd_sb[:, q, :], in_=gps[:, H:H + 2 * HEADS])
    ctxB.close()

    # ---------------- phase D: attention ----------------
    ctxD = ExitStack()
    srcps = ctxD.enter_context(tc.tile_pool(name="srcps", bufs=2, space="PSUM"))
    attp = ctxD.enter_context(tc.tile_pool(name="attp", bufs=4, space="PSUM"))
    psTr = ctxD.enter_context(tc.tile_pool(name="psTrD", bufs=2, space="PSUM"))
    srcbp = ctxD.enter_context(tc.tile_pool(name="srcbp", bufs=2))
    scp = ctxD.enter_context(tc.tile_pool(name="scp", bufs=2))
    ptp = ctxD.enter_context(tc.tile_pool(name="ptp", bufs=2))
    ptlp = ctxD.enter_context(tc.tile_pool(name="ptlp", bufs=2))
    t2p = ctxD.enter_context(tc.tile_pool(name="t2p", bufs=2))

    NQ = 4                      # quarters for leaky/exp pipelining
    QW = NCH // NQ              # j-chunks per quarter (2)

    # software pipeline: emit head h's scores, then head h-1's tail
    state = {}

    def emit_scores(h):
        # srcb[p, q] = src_h[q] for all p, via replicated-column matmul
        srcb = srcbp.tile([128, N], BF, tag="srcb")
        for half in range(2):
            sps = srcps.tile([128, 512], F32, tag="sbc")
            for c in range(CCH):
                nc.tensor.matmul(
                    sps[:],
                    lhsT=wsr_sb[:, h, c, :],
                    rhs=hT_sb[:, c, half * 512:half * 512 + 512],
                    start=(c == 0), stop=(c == CCH - 1))
            nc.vector.tensor_copy(
                out=srcb[:, half * 512:half * 512 + 512], in_=sps[:])

        # ScalarE Prelu path first (chunks 3..7; dst rides the Prelu bias),
        # DVE leaky path for chunks 0..2.
        NSV = 3                   # chunks < NSV take the DVE path
        s1 = scp.tile([128, NCH, N], BF, tag="s1")
        pt = ptp.tile([128, NCH, N], BF, tag="pt")
        ptl = ptlp.tile([128, NCH, N], BF, tag="ptl")
        for j in range(NSV, NCH):
            nc.vector.tensor_tensor(
                out=s1[:, j, :], in0=srcb[:], in1=mask_sb[:, j, :],
                op=mybir.AluOpType.add)
            if sim_compat:
                nc.vector.tensor_scalar_add(
                    out=s1[:, j, :], in0=s1[:, j, :],
                    scalar1=sd_sb[:, j, HEADS + h:HEADS + h + 1])
                nc.vector.scalar_tensor_tensor(
                    out=ptl[:, j, :], in0=s1[:, j, :], scalar=0.2,
                    in1=s1[:, j, :],
                    op0=mybir.AluOpType.mult, op1=mybir.AluOpType.max)
            else:
                nc.scalar.activation(
                    out=ptl[:, j, :], in_=s1[:, j, :],
                    func=mybir.ActivationFunctionType.Prelu,
                    bias=sd_sb[:, j, HEADS + h:HEADS + h + 1],
                    scale=1.0, alpha=0.2)
            if j % 2 == 1 and j >= NSV + 1:
                nc.scalar.activation(
                    out=pt[:, j - 1:j + 1, :], in_=ptl[:, j - 1:j + 1, :],
                    func=mybir.ActivationFunctionType.Exp,
                    bias=zero_sb[:], scale=1.0)

        # DVE leaky path
        for j in range(NSV):
            nc.vector.tensor_scalar_add(
                out=s1[:, j, :], in0=srcb[:],
                scalar1=sd_sb[:, j, HEADS + h:HEADS + h + 1])
        nc.vector.tensor_tensor(
            out=s1[:, 0:NSV, :], in0=s1[:, 0:NSV, :], in1=mask_sb[:, 0:NSV, :],
            op=mybir.AluOpType.add)
        if sim_compat:
            nc.vector.scalar_tensor_tensor(
                out=ptl[:, 0:NSV, :], in0=s1[:, 0:NSV, :], scalar=0.2,
                in1=s1[:, 0:NSV, :],
                op0=mybir.AluOpType.mult, op1=mybir.AluOpType.max)
        else:
            t2 = t2p.tile([128, NSV, N], BF, tag="t2")
            nc.vector.tensor_scalar_mul(
                out=t2[:], in0=s1[:, 0:NSV, :], scalar1=0.2)
            nc.vector.tensor_tensor(
                out=ptl[:, 0:NSV, :], in0=s1[:, 0:NSV, :], in1=t2[:],
                op=mybir.AluOpType.max)
        nc.scalar.activation(out=pt[:, 0:NSV + 1, :], in_=ptl[:, 0:NSV + 1, :],
                             func=mybir.ActivationFunctionType.Exp,
                             bias=zero_sb[:], scale=1.0)

        # hp accumulation, prelu-path chunks first (their pt is ready first)
        hp0 = attp.tile([D + 1, 512], F32, tag="hpT")
        hp1 = attp.tile([D + 1, 512], F32, tag="hpT")
        jorder = (4, 5, 6, 7, 0, 1, 2, 3)
        for ji, j in enumerate(jorder):
            for half, hps_ in ((0, hp0), (1, hp1)):
                nc.tensor.matmul(
                    hps_[:],
                    lhsT=wh_all[:, j, h * (D + 1):(h + 1) * (D + 1)],
                    rhs=pt[:, j, half * 512:half * 512 + 512],
                    start=(ji == 0), stop=(ji == NCH - 1))
        state[h] = (hp0, hp1)

    def emit_tail(h, last=False):
        hp0, hp1 = state.pop(h)
        hpt = work.tile([D + 1, N], F32, tag="hpt")
        nc.vector.tensor_copy(out=hpt[:, 0:512], in_=hp0[:])
        nc.scalar.copy(out=hpt[:, 512:N], in_=hp1[:])
        for q in range(NCH):
            tq = psTr.tile([128, D + 1], F32, tag="trq")
            nc.tensor.transpose(out=tq[:], in_=hpt[:, q * 128:(q + 1) * 128],
                                identity=ident_f32[0:D + 1, 0:D + 1])
            rd = statp.tile([128, 1], F32, tag="rd")
            nc.vector.reciprocal(out=rd[:], in_=tq[:, D:D + 1])
            nc.vector.tensor_scalar_mul(
                out=hp_all[:, q, h * D:(h + 1) * D],
                in0=tq[:, 0:D], scalar1=rd[:])
            if last:
                st = statp.tile([128, 6], F32, tag="bn2")
                nc.vector.bn_stats(out=st[:], in_=hp_all[:, q, :])
                nc.vector.bn_aggr(out=mv2[:, q, :], in_=st[:])

    for h in range(HEADS):
        emit_scores(h)
        if h > 0:
            emit_tail(h - 1)
    emit_tail(HEADS - 1, last=True)

    ctxD.close()
    # ---------------- phase E: ln2 + out (stats done in last tail) --------
    nc.scalar.activation(out=rstd2[:], in_=mv2[:, :, 1],
                         func=mybir.ActivationFunctionType.Ln, bias=eps_sb[:],
                         scale=1.0)
    nc.scalar.activation(out=rstd2[:], in_=rstd2[:],
                         func=mybir.ActivationFunctionType.Exp, bias=zero_sb[:],
                         scale=-0.5)
    for q in range(NCH):
        ot = outp.tile([128, H], F32, tag="out")
        nc.vector.tensor_scalar(
            out=ot[:], in0=hp_all[:, q, :],
            scalar1=mv2[:, q, 0:1], scalar2=rstd2[:, q:q + 1],
            op0=mybir.AluOpType.subtract, op1=mybir.AluOpType.mult)
        nc.sync.dma_start(out=out_d[q * 128:(q + 1) * 128, :], in_=ot[:])


def _prep(inputs):
    """Host-side parameter folding. Returns per-core input maps."""
    bf16 = ml_dtypes.bfloat16
    f = lambda a: np.ascontiguousarray(np.asarray(a, np.float32))

    x = f(inputs["x"])
    adj = np.asarray(inputs["adj"])
    conv_w = f(inputs["conv_w"]); conv_b = f(inputs["conv_b"])
    bn_g = f(inputs["bn_g"]); bn_b = f(inputs["bn_b"])
    fw = f(inputs["fusion_weight"])
    W_low = f(inputs["W_low"]); b_low = f(inputs["b_low"])
    W_high = f(inputs["W_high"]); b_high = f(inputs["b_high"])
    ln1_g = f(inputs["ln1_g"]); ln1_b = f(inputs["ln1_b"])
    gat_W = f(inputs["gat_W"])
    a_src = f(inputs["a_src"]); a_dst = f(inputs["a_dst"])
    ln2_g = f(inputs["ln2_g"]); ln2_b = f(inputs["ln2_b"])

    trivial = dict(
        b_low=np.allclose(b_low, 0), b_high=np.allclose(b_high, 0),
        ln1=np.allclose(ln1_g, 1) and np.allclose(ln1_b, 0),
        ln2=np.allclose(ln2_g, 1) and np.allclose(ln2_b, 0))
    if not all(trivial.values()):
        raise NotImplementedError(f"non-trivial affine params: {trivial}")

    alpha = np.exp(fw - fw.max()); alpha /= alpha.sum()
    gprime = bn_g / np.float32(np.sqrt(1.0 + EPS))          # [S,H]
    bconv = conv_b * gprime + bn_b                           # [S,H]
    # Wt[i,k,cin,cout] = conv_w[i,cout,cin,k]*gprime[i,cout]
    Wt = np.transpose(conv_w, (0, 3, 2, 1)) * gprime[:, None, None, :]
    # [S,K,cin,H] -> [S,K,CCH,128,H] -> [S*K*CCH,128,H]
    Wt = Wt.reshape(S, K, CCH, 128, H).reshape(S * K * CCH, 128, H)
    # bconv laid out [128, S*CCH]: column i*CCH+c holds channels c*128..c*128+127
    bconv_t = bconv.reshape(S, CCH, 128).transpose(2, 0, 1).reshape(128, S * CCH)

    WlowA = (alpha[:, None, None] * W_low[None]).reshape(S, CCH, 128, BOT)
    WlowA = WlowA.reshape(S * CCH, 128, BOT)

    G = np.zeros((H, H + 2 * HEADS), np.float32)
    for h in range(HEADS):
        G[:, h * D:(h + 1) * D] = gat_W[h]
        G[:, H + h] = gat_W[h] @ a_src[h]
        G[:, H + HEADS + h] = gat_W[h] @ a_dst[h]
    Gr = G.reshape(CCH, 128, H + 2 * HEADS)

    maskT = np.where(adj.T > 0, np.float32(0.0), np.float32(NEG))
    maskTr = maskT.reshape(NCH, 128, N)

    # wsrcrep[h, c, :, j] = (gat_W[h] @ a_src[h])[c*128 + :]  (all 128 cols equal)
    wsrc = np.stack([gat_W[h] @ a_src[h] for h in range(HEADS)])  # [HEADS, H]
    wsrcrep = np.repeat(
        wsrc.reshape(HEADS, CCH, 128, 1), 128, axis=3).astype(np.float32)

    S_HEADS = tuple(h for h in range(HEADS) if h not in V_PATH)
    sel = np.zeros((33, 2, len(S_HEADS), 2), np.float32)
    for hi, h in enumerate(S_HEADS):
        sel[HEADS + h, 0, hi, 0] = 1.0   # csdL row0 = dst_h
        sel[32, 1, hi, 0] = 1.0          # csdL row1 = ones
        sel[32, 0, hi, 1] = 1.0          # csdR row0 = ones
        sel[h, 1, hi, 1] = 1.0           # csdR row1 = src_h
    shared = {
        "csdsel": sel.astype(bf16),
        "wt": np.ascontiguousarray(Wt.transpose(1, 0, 2)).astype(bf16),
        "bconv": np.ascontiguousarray(bconv_t),
        "wlow": np.ascontiguousarray(WlowA.transpose(1, 0, 2)).astype(bf16),
        "whigh": W_high.astype(bf16),
        "gmat": np.ascontiguousarray(Gr.transpose(1, 0, 2)).astype(bf16),
        "maskT": np.ascontiguousarray(maskTr.transpose(1, 0, 2)).astype(bf16),
        "wsrcrep": np.ascontiguousarray(
            wsrcrep.transpose(2, 0, 1, 3)).astype(bf16),
    }
    in_maps = []
    for b in range(B):
        xt = np.ascontiguousarray(x[b].T)                    # [H, N]
        m = dict(shared)
        m["xt"] = np.ascontiguousarray(
            xt.reshape(CCH, 128, N).transpose(1, 0, 2)).astype(bf16)
        m["xres"] = np.ascontiguousarray(
            x[b].reshape(NCH, 128, H).transpose(1, 0, 2))
        in_maps.append(m)
    return in_maps, trivial


def kernel(**inputs) -> np.ndarray:
    in_maps, trivial = _prep(inputs)
    key = "k"
    if key not in _CACHED:
        _CACHED[key] = _build(trivial)
    nc = _CACHED[key]
    res = run_bass_kernel_spmd(nc, in_maps, list(range(B)))
    out = np.stack([res.results[i]["out"] for i in range(B)], axis=0)
    return out.astype(np.float32)


if __name__ == "__main__":
    import reference
    inputs = {k: np.asarray(v) for k, v in reference.setup_inputs().items()}
    got = kernel(**inputs)
    print("kernel output", got.shape, got.dtype)


# revision 32
# speedup vs baseline: 1.2324x; 1.2324x over previous
"""Trainium2 Bass kernel for nn_LocationAwareMSAGAT_Net.

Strategy: data-parallel over batch B=8 across the 8 NeuronCores (one batch
element per core); all parameters replicated.  Per core:

  phase A: multi-scale dilated conv (as 24 shifted matmuls, bf16) + BN fold
           + SiLU (ScalarE, conv bias folded into activation bias)
  phase B: bottleneck (alpha folded into W_low; accumulated in PSUM over
           scales) -> W_high -> +residual -> LayerNorm1 -> transpose (PE)
  phase C: GAT projections: one matmul computes Wh for all heads plus
           src/dst attention logits (gat_W@a_src / gat_W@a_dst appended as
           extra columns)
  phase D: attention, computed transposed (P^T[m,q] tiles), per head:
           srcb = broadcast src (replicated-column matmul, PE)
           s1   = srcb + dst         (DVE tensor_scalar, per-partition dst)
           s1  += maskNEG            (DVE tensor_tensor, in halves)
           leaky-relu + exp, two flavors to balance engines:
             V-path heads: t2 = 0.2*s1 (DVE 4x), ptl = max(s1,t2) (DVE 2x),
                           pt = exp(ptl) (ScalarE)
             S-path heads: ptl = Lrelu(s1, alpha=0.2) (ScalarE),
                           pt = exp(ptl) (ScalarE)
           hp^T = [Wh_h | ones]^T @ P^T accumulated in PSUM over m-chunks
           (ones column yields softmax denominators)
           PE-transpose back, divide rows by denominator
  phase E: LayerNorm2 -> DMA out

Everything on the PE is bf16 with fp32 PSUM accumulation.
"""

import os
import numpy as np
import ml_dtypes
from contextlib import ExitStack

import concourse.bass as bass
import concourse.tile as tile
from concourse import bacc, mybir
from concourse.bass_utils import run_bass_kernel_spmd
from concourse.masks import make_identity

BF = mybir.dt.bfloat16
F32 = mybir.dt.float32
EPS = 1e-5
NEG = -1e9

B, N, H = 8, 1024, 256
S, K, HEADS = 4, 3, 4
D = H // HEADS          # 64
NCH = N // 128          # 8 chunks of 128
CCH = H // 128          # 2 channel chunks
BOT = 8                 # bottleneck dim

V_PATH = (0, 2)         # heads whose leaky-relu runs on DVE; rest on ScalarE
HEAD_ORDER = (1, 0, 3, 2)  # S-path heads early so the tail head is V-path
S_HEADS = tuple(h for h in range(HEADS) if h not in V_PATH)

_CACHED = {}


def _build(trivial: dict) -> bass.Bass:
    nc = bacc.Bacc("TRN2", target_bir_lowering=False, debug=False,
                   num_devices=B)

    xt_d = nc.declare_dram_parameter("xt", [128, CCH, N], BF, isOutput=False)
    xres_d = nc.declare_dram_parameter("xres", [128, NCH, H], F32, isOutput=False)
    wt_d = nc.declare_dram_parameter("wt", [128, S * K * CCH, H], BF, isOutput=False)
    bconv_d = nc.declare_dram_parameter("bconv", [128, S * CCH], F32, isOutput=False)
    wlow_d = nc.declare_dram_parameter("wlow", [128, S * CCH, BOT], BF, isOutput=False)
    whigh_d = nc.declare_dram_parameter("whigh", [BOT, H], BF, isOutput=False)
    g_d = nc.declare_dram_parameter("gmat", [128, CCH, H + 2 * HEADS], BF,
                                    isOutput=False)
    mask_d = nc.declare_dram_parameter("maskT", [128, NCH, N], BF, isOutput=False)
    wsr_d = nc.declare_dram_parameter("wsrcrep", [128, HEADS, CCH, 128], BF,
                                      isOutput=False)
    sel_d = nc.declare_dram_parameter("csdsel", [33, 2, 2, 2], BF,
                                      isOutput=False)
    out_d = nc.declare_dram_parameter("out", [N, H], F32, isOutput=True)

    with tile.TileContext(nc) as tc:
        with ExitStack() as ctx:
            _body(ctx, tc, xt_d, xres_d, wt_d, bconv_d, wlow_d, whigh_d, g_d,
                  mask_d, wsr_d, sel_d, out_d)
    nc.compile()
    return nc


def _body(ctx, tc, xt_d, xres_d, wt_d, bconv_d, wlow_d, whigh_d, g_d,
          mask_d, wsr_d, sel_d, out_d):
    nc = tc.nc
    consts = ctx.enter_context(tc.tile_pool(name="consts", bufs=1))
    work = ctx.enter_context(tc.tile_pool(name="work", bufs=3))
    statp = ctx.enter_context(tc.tile_pool(name="stats", bufs=4))
    outp = ctx.enter_context(tc.tile_pool(name="outp", bufs=3))

    ctxA = ExitStack()
    apool = ctxA.enter_context(tc.tile_pool(name="apool", bufs=1))

    # ---------------- constants / inputs into SBUF ----------------
    # sync queue: phase-A-critical bytes, in consumption order.
    # gpsimd (software DGE) queue: everything needed later (mask last).
    xpad = apool.tile([128, CCH, N + 16], BF, tag="xpad")
    nc.vector.memset(xpad[:, :, 0:8], 0.0)
    nc.vector.memset(xpad[:, :, N + 8:N + 16], 0.0)
    for c in range(CCH):
        nc.sync.dma_start(out=xpad[:, c, 8:8 + N], in_=xt_d[:, c, :])

    bconv_sb = apool.tile([128, S * CCH], F32, tag="bconv")
    nc.sync.dma_start(out=bconv_sb[:], in_=bconv_d[:])

    wt_sbs = []
    for i in range(S):
        w = apool.tile([128, K * CCH, H], BF, tag=f"wt{i}")
        nc.sync.dma_start(out=w[:],
                          in_=wt_d[:, i * K * CCH:(i + 1) * K * CCH, :])
        wt_sbs.append(w)

    wlow_sb = apool.tile([128, S * CCH, BOT], BF, tag="wlow")
    nc.sync.dma_start(out=wlow_sb[:], in_=wlow_d[:])

    whigh_sb = consts.tile([BOT, H], BF, tag="whigh")
    nc.sync.dma_start(out=whigh_sb[:], in_=whigh_d[:])

    xres_sb = consts.tile([128, NCH, H], F32, tag="xres")
    nc.sync.dma_start(out=xres_sb[:], in_=xres_d[:])

    g_sb = consts.tile([128, CCH, H + 2 * HEADS], BF, tag="gmat")
    nc.sync.dma_start(out=g_sb[:], in_=g_d[:])

    wsr_sb = consts.tile([128, HEADS, CCH, 128], BF, tag="wsr")
    nc.sync.dma_start(out=wsr_sb[:], in_=wsr_d[:])

    mask_sb = consts.tile([128, NCH, N], BF, tag="mask")
    nc.sync.dma_start(out=mask_sb[:], in_=mask_d[:])

    ident_bf = consts.tile([128, 128], BF, tag="idbf")
    make_identity(nc, ident_bf[:])
    ident_f32 = consts.tile([128, 128], F32, tag="idf32")
    make_identity(nc, ident_f32[:])
    eps_sb = consts.tile([128, 1], F32, tag="eps")
    nc.vector.memset(eps_sb[:], EPS)
    zero_sb = consts.tile([128, 1], F32, tag="zero")
    nc.vector.memset(zero_sb[:], 0.0)

    # persistent intermediates
    fused_sb = apool.tile([128, S, CCH, N], BF, tag="fused")
    lowT_sb = consts.tile([BOT, N], BF, tag="lowT")
    h_all = consts.tile([128, NCH, H], F32, tag="h_all")
    mv1 = consts.tile([128, NCH, 2], F32, tag="mv1")
    rstd1 = consts.tile([128, NCH], F32, tag="rstd1")
    hT_sb = consts.tile([128, CCH, N], BF, tag="hT")
    wh_all = consts.tile([128, NCH, HEADS * (D + 1)], BF, tag="wh")
    nc.vector.memset(
        wh_all[:].rearrange("p j (h x) -> p j h x", x=D + 1)[:, :, :, D], 1.0)
    sd_sb = consts.tile([128, NCH, 2 * HEADS], F32, tag="sd")
    hp_all = consts.tile([128, NCH, H], F32, tag="hp")
    mv2 = consts.tile([128, NCH, 2], F32, tag="mv2")
    rstd2 = consts.tile([128, NCH], F32, tag="rstd2")

    sim_compat = os.environ.get("BASS_SIM_COMPAT", "0") == "1"

    # ---------------- phase A: conv + silu (nch-outer so phase B of each
    # N-half overlaps the other half's conv matmuls) ----------------
    ctxAB = ExitStack()
    psB = ctxAB.enter_context(tc.tile_pool(name="psB", bufs=2, space="PSUM"))
    convp = ctxA.enter_context(tc.tile_pool(name="convp", bufs=4, space="PSUM"))
    lowp = ctxA.enter_context(tc.tile_pool(name="lowp", bufs=2, space="PSUM"))
    for nch in range(2):
        for i in range(S):
            for cout in range(CCH):
                ps = convp.tile([128, 512], F32, tag="conv")
                dil = 2 ** i
                first = True
                for c in range(CCH):
                    for k in range(K):
                        sh = (k - 1) * dil
                        t = k * CCH + c
                        nc.tensor.matmul(
                            ps[:],
                            lhsT=wt_sbs[i][:, t, cout * 128:(cout + 1) * 128],
                            rhs=xpad[:, c, 8 + sh + nch * 512:
                                     8 + sh + nch * 512 + 512],
                            start=first, stop=(c == CCH - 1 and k == K - 1))
                        first = False
                dst = fused_sb[:, i, cout, nch * 512:nch * 512 + 512]
                bias_ap = bconv_sb[:, i * CCH + cout:i * CCH + cout + 1]
                if sim_compat:
                    # CoreSim has no Silu: sigmoid + fused (ps+b)*sig on DVE
                    sg = work.tile([128, 512], F32, tag="sg")
                    nc.scalar.activation(
                        out=sg[:], in_=ps[:],
                        func=mybir.ActivationFunctionType.Sigmoid,
                        bias=bias_ap, scale=1.0)
                    nc.vector.scalar_tensor_tensor(
                        out=dst, in0=ps[:], scalar=bias_ap, in1=sg[:],
                        op0=mybir.AluOpType.add, op1=mybir.AluOpType.mult)
                else:
                    nc.scalar.activation(
                        out=dst, in_=ps[:],
                        func=mybir.ActivationFunctionType.Silu,
                        bias=bias_ap, scale=1.0)

        # A2 for this half: lowT = sum_i (a_i W_low)^T @ silu_i
        lps = lowp.tile([BOT, 512], F32, tag="low")
        first = True
        for i in range(S):
            for c in range(CCH):
                nc.tensor.matmul(
                    lps[:],
                    lhsT=wlow_sb[:, i * CCH + c, :],
                    rhs=fused_sb[:, i, c, nch * 512:nch * 512 + 512],
                    start=first, stop=(i == S - 1 and c == CCH - 1))
                first = False
        nc.vector.tensor_copy(out=lowT_sb[:, nch * 512:nch * 512 + 512],
                              in_=lps[:])

        # B part 1 for this half: high + residual + stats
        for q in range(nch * 4, nch * 4 + 4):
            hps = psB.tile([128, H], F32, tag="high")
            nc.tensor.matmul(hps[:], lhsT=lowT_sb[:, q * 128:(q + 1) * 128],
                             rhs=whigh_sb[:], start=True, stop=True)
            nc.vector.tensor_add(h_all[:, q, :], hps[:], xres_sb[:, q, :])
            st = statp.tile([128, 6], F32, tag="bn1")
            nc.vector.bn_stats(out=st[:], in_=h_all[:, q, :])
            nc.vector.bn_aggr(out=mv1[:, q, :], in_=st[:])
    ctxA.close()
    ctxAB.close()

    # ---------------- phase B2: ln1 + transpose, pipelined into C ----------
    ctxB = ExitStack()
    psTrB = ctxB.enter_context(tc.tile_pool(name="psTrB", bufs=3, space="PSUM"))
    psC = ctxB.enter_context(tc.tile_pool(name="psC", bufs=2, space="PSUM"))
    # rstd1 = exp(-0.5 * ln(var + eps)), batched: Ln and Exp live in
    # different walrus table sets, so interleaving them thrashes
    # ACT_TABLE_LOAD (~1.3us each).  Two batched calls -> two loads total.
    nc.scalar.activation(out=rstd1[:], in_=mv1[:, :, 1],
                         func=mybir.ActivationFunctionType.Ln,
                         bias=eps_sb[:], scale=1.0)
    nc.scalar.activation(out=rstd1[:], in_=rstd1[:],
                         func=mybir.ActivationFunctionType.Exp,
                         bias=zero_sb[:], scale=-0.5)
    for q in range(NCH):
        hn = work.tile([128, H], BF, tag="hn")
        nc.vector.tensor_scalar(
            out=hn[:], in0=h_all[:, q, :],
            scalar1=mv1[:, q, 0:1], scalar2=rstd1[:, q:q + 1],
            op0=mybir.AluOpType.subtract, op1=mybir.AluOpType.mult)
        for c in range(CCH):
            tp = psTrB.tile([128, 128], BF, tag="trh")
            nc.tensor.transpose(out=tp[:],
                                in_=hn[:, c * 128:(c + 1) * 128],
                                identity=ident_bf[:])
            nc.scalar.copy(out=hT_sb[:, c, q * 128:(q + 1) * 128],
                           in_=tp[:])
        # ------- phase C for this chunk: GAT projections -------
        gps = psC.tile([128, H + 2 * HEADS], F32, tag="gat")
        for c in range(CCH):
            nc.tensor.matmul(gps[:], lhsT=hT_sb[:, c, q * 128:(q + 1) * 128],
                             rhs=g_sb[:, c, :], start=(c == 0),
                             stop=(c == CCH - 1))
        whj = wh_all[:, q, :].rearrange("p (h x) -> p h x", x=D + 1)
        nc.vector.tensor_copy(
            out=whj[:, :, 0:D],
            in_=gps[:, 0:H].rearrange("p (h x) -> p h x", x=D))
        nc.vector.tensor_copy(out=sd_sb[:, q, :], in_=gps[:, H:H + 2 * HEADS])
    ctxB.close()

    # ---------------- phase D: attention ----------------
    ctxD = ExitStack()
    srcps = ctxD.enter_context(tc.tile_pool(name="srcps", bufs=2, space="PSUM"))
    attp = ctxD.enter_context(tc.tile_pool(name="attp", bufs=4, space="PSUM"))
    psTr = ctxD.enter_context(tc.tile_pool(name="psTrD", bufs=2, space="PSUM"))
    srcbp = ctxD.enter_context(tc.tile_pool(name="srcbp", bufs=2))
    scp = ctxD.enter_context(tc.tile_pool(name="scp", bufs=2))
    ptp = ctxD.enter_context(tc.tile_pool(name="ptp", bufs=2))
    ptlp = ctxD.enter_context(tc.tile_pool(name="ptlp", bufs=2))
    t2p = ctxD.enter_context(tc.tile_pool(name="t2p", bufs=2))

    NQ = 4                      # quarters for leaky/exp pipelining
    QW = NCH // NQ              # j-chunks per quarter (2)

    # software pipeline: emit head h's scores, then head h-1's tail
    state = {}

    def emit_scores(h):
        # srcb[p, q] = src_h[q] for all p, via replicated-column matmul
        srcb = srcbp.tile([128, N], BF, tag="srcb")
        for half in range(2):
            sps = srcps.tile([128, 512], F32, tag="sbc")
            for c in range(CCH):
                nc.tensor.matmul(
                    sps[:],
                    lhsT=wsr_sb[:, h, c, :],
                    rhs=hT_sb[:, c, half * 512:half * 512 + 512],
                    start=(c == 0), stop=(c == CCH - 1))
            nc.vector.tensor_copy(
                out=srcb[:, half * 512:half * 512 + 512], in_=sps[:])

        # ScalarE Prelu path first (chunks 3..7; dst rides the Prelu bias),
        # DVE leaky path for chunks 0..2.
        NSV = 3                   # chunks < NSV take the DVE path
        s1 = scp.tile([128, NCH, N], BF, tag="s1")
        pt = ptp.tile([128, NCH, N], BF, tag="pt")
        ptl = ptlp.tile([128, NCH, N], BF, tag="ptl")
        for j in range(NSV, NCH):
            nc.vector.tensor_tensor(
                out=s1[:, j, :], in0=srcb[:], in1=mask_sb[:, j, :],
                op=mybir.AluOpType.add)
            if sim_compat:
                nc.vector.tensor_scalar_add(
                    out=s1[:, j, :], in0=s1[:, j, :],
                    scalar1=sd_sb[:, j, HEADS + h:HEADS + h + 1])
                nc.vector.scalar_tensor_tensor(
                    out=ptl[:, j, :], in0=s1[:, j, :], scalar=0.2,
                    in1=s1[:, j, :],
                    op0=mybir.AluOpType.mult, op1=mybir.AluOpType.max)
            else:
                nc.scalar.activation(
                    out=ptl[:, j, :], in_=s1[:, j, :],
                    func=mybir.ActivationFunctionType.Prelu,
                    bias=sd_sb[:, j, HEADS + h:HEADS + h + 1],
                    scale=1.0, alpha=0.2)
            if j % 2 == 1 and j >= NSV + 1:
                nc.scalar.activation(
                    out=pt[:, j - 1:j + 1, :], in_=ptl[:, j - 1:j + 1, :],
                    func=mybir.ActivationFunctionType.Exp,
                    bias=zero_sb[:], scale=1.0)

        # DVE leaky path
        for j in range(NSV):
            nc.vector.tensor_scalar_add(
                out=s1[:, j, :], in0=srcb[:],
                scalar1=sd_sb[:, j, HEADS + h:HEADS + h + 1])
        nc.vector.tensor_tensor(
            out=s1[:, 0:NSV, :], in0=s1[:, 0:NSV, :], in1=mask_sb[:, 0:NSV, :],
            op=mybir.AluOpType.add)
        if sim_compat:
            nc.vector.scalar_tensor_tensor(
                out=ptl[:, 0:NSV, :], in0=s1[:, 0:NSV, :], scalar=0.2,
                in1=s1[:, 0:NSV, :],
                op0=mybir.AluOpType.mult, op1=mybir.AluOpType.max)
        else:
            t2 = t2p.tile([128, NSV, N], BF, tag="t2")
            nc.vector.tensor_scalar_mul(
                out=t2[:], in0=s1[:, 0:NSV, :], scalar1=0.2)
            nc.vector.tensor_tensor(
                out=ptl[:, 0:NSV, :], in0=s1[:, 0:NSV, :], in1=t2[:],
                op=mybir.AluOpType.max)
        nc.scalar.activation(out=pt[:, 0:2, :], in_=ptl[:, 0:2, :],
                             func=mybir.ActivationFunctionType.Exp,
                             bias=zero_sb[:], scale=1.0)
        nc.scalar.activation(out=pt[:, 2:NSV + 1, :], in_=ptl[:, 2:NSV + 1, :],
                             func=mybir.ActivationFunctionType.Exp,
                             bias=zero_sb[:], scale=1.0)

        # hp accumulation, prelu-path chunks first (their pt is ready first)
        hp0 = attp.tile([D + 1, 512], F32, tag="hpT")
        hp1 = attp.tile([D + 1, 512], F32, tag="hpT")
        jorder = (4, 5, 6, 7, 0, 1, 2, 3)
        for ji, j in enumerate(jorder):
            for half, hps_ in ((0, hp0), (1, hp1)):
                nc.tensor.matmul(
                    hps_[:],
                    lhsT=wh_all[:, j, h * (D + 1):(h + 1) * (D + 1)],
                    rhs=pt[:, j, half * 512:half * 512 + 512],
                    start=(ji == 0), stop=(ji == NCH - 1))
        state[h] = (hp0, hp1)

    def emit_tail(h, last=False):
        hp0, hp1 = state.pop(h)
        hpt = work.tile([D + 1, N], F32, tag="hpt")
        nc.vector.tensor_copy(out=hpt[:, 0:512], in_=hp0[:])
        nc.vector.tensor_copy(out=hpt[:, 512:N], in_=hp1[:])
        for q in range(NCH):
            tq = psTr.tile([128, D + 1], F32, tag="trq")
            nc.tensor.transpose(out=tq[:], in_=hpt[:, q * 128:(q + 1) * 128],
                                identity=ident_f32[0:D + 1, 0:D + 1])
            rd = statp.tile([128, 1], F32, tag="rd")
            nc.vector.reciprocal(out=rd[:], in_=tq[:, D:D + 1])
            nc.vector.tensor_scalar_mul(
                out=hp_all[:, q, h * D:(h + 1) * D],
                in0=tq[:, 0:D], scalar1=rd[:])
            if last:
                st = statp.tile([128, 6], F32, tag="bn2")
                nc.vector.bn_stats(out=st[:], in_=hp_all[:, q, :])
                nc.vector.bn_aggr(out=mv2[:, q, :], in_=st[:])

    for h in range(HEADS):
        emit_scores(h)
        if h > 0:
            emit_tail(h - 1)
    emit_tail(HEADS - 1, last=True)

    ctxD.close()
    # ---------------- phase E: ln2 + out (stats done in last tail) --------
    nc.scalar.activation(out=rstd2[:], in_=mv2[:, :, 1],
                         func=mybir.ActivationFunctionType.Ln, bias=eps_sb[:],
                         scale=1.0)
    nc.scalar.activation(out=rstd2[:], in_=rstd2[:],
                         func=mybir.ActivationFunctionType.Exp, bias=zero_sb[:],
                         scale=-0.5)
    for q in range(NCH):
        ot = outp.tile([128, H], F32, tag="out")
        nc.vector.tensor_scalar(
            out=ot[:], in0=hp_all[:, q, :],
            scalar1=mv2[:, q, 0:1], scalar2=rstd2[:, q:q + 1],
            op0=mybir.AluOpType.subtract, op1=mybir.AluOpType.mult)
        nc.sync.dma_start(out=out_d[q * 128:(q + 1) * 128, :], in_=ot[:])


def _prep(inputs):
    """Host-side parameter folding. Returns per-core input maps."""
    bf16 = ml_dtypes.bfloat16
    f = lambda a: np.ascontiguousarray(np.asarray(a, np.float32))

    x = f(inputs["x"])
    adj = np.asarray(inputs["adj"])
    conv_w = f(inputs["conv_w"]); conv_b = f(inputs["conv_b"])
    bn_g = f(inputs["bn_g"]); bn_b = f(inputs["bn_b"])
    fw = f(inputs["fusion_weight"])
    W_low = f(inputs["W_low"]); b_low = f(inputs["b_low"])
    W_high = f(inputs["W_high"]); b_high = f(inputs["b_high"])
    ln1_g = f(inputs["ln1_g"]); ln1_b = f(inputs["ln1_b"])
    gat_W = f(inputs["gat_W"])
    a_src = f(inputs["a_src"]); a_dst = f(inputs["a_dst"])
    ln2_g = f(inputs["ln2_g"]); ln2_b = f(inputs["ln2_b"])

    trivial = dict(
        b_low=np.allclose(b_low, 0), b_high=np.allclose(b_high, 0),
        ln1=np.allclose(ln1_g, 1) and np.allclose(ln1_b, 0),
        ln2=np.allclose(ln2_g, 1) and np.allclose(ln2_b, 0))
    if not all(trivial.values()):
        raise NotImplementedError(f"non-trivial affine params: {trivial}")

    alpha = np.exp(fw - fw.max()); alpha /= alpha.sum()
    gprime = bn_g / np.float32(np.sqrt(1.0 + EPS))          # [S,H]
    bconv = conv_b * gprime + bn_b                           # [S,H]
    # Wt[i,k,cin,cout] = conv_w[i,cout,cin,k]*gprime[i,cout]
    Wt = np.transpose(conv_w, (0, 3, 2, 1)) * gprime[:, None, None, :]
    # [S,K,cin,H] -> [S,K,CCH,128,H] -> [S*K*CCH,128,H]
    Wt = Wt.reshape(S, K, CCH, 128, H).reshape(S * K * CCH, 128, H)
    # bconv laid out [128, S*CCH]: column i*CCH+c holds channels c*128..c*128+127
    bconv_t = bconv.reshape(S, CCH, 128).transpose(2, 0, 1).reshape(128, S * CCH)

    WlowA = (alpha[:, None, None] * W_low[None]).reshape(S, CCH, 128, BOT)
    WlowA = WlowA.reshape(S * CCH, 128, BOT)

    G = np.zeros((H, H + 2 * HEADS), np.float32)
    for h in range(HEADS):
        G[:, h * D:(h + 1) * D] = gat_W[h]
        G[:, H + h] = gat_W[h] @ a_src[h]
        G[:, H + HEADS + h] = gat_W[h] @ a_dst[h]
    Gr = G.reshape(CCH, 128, H + 2 * HEADS)

    maskT = np.where(adj.T > 0, np.float32(0.0), np.float32(NEG))
    maskTr = maskT.reshape(NCH, 128, N)

    # wsrcrep[h, c, :, j] = (gat_W[h] @ a_src[h])[c*128 + :]  (all 128 cols equal)
    wsrc = np.stack([gat_W[h] @ a_src[h] for h in range(HEADS)])  # [HEADS, H]
    wsrcrep = np.repeat(
        wsrc.reshape(HEADS, CCH, 128, 1), 128, axis=3).astype(np.float32)

    S_HEADS = tuple(h for h in range(HEADS) if h not in V_PATH)
    sel = np.zeros((33, 2, len(S_HEADS), 2), np.float32)
    for hi, h in enumerate(S_HEADS):
        sel[HEADS + h, 0, hi, 0] = 1.0   # csdL row0 = dst_h
        sel[32, 1, hi, 0] = 1.0          # csdL row1 = ones
        sel[32, 0, hi, 1] = 1.0          # csdR row0 = ones
        sel[h, 1, hi, 1] = 1.0           # csdR row1 = src_h
    shared = {
        "csdsel": sel.astype(bf16),
        "wt": np.ascontiguousarray(Wt.transpose(1, 0, 2)).astype(bf16),
        "bconv": np.ascontiguousarray(bconv_t),
        "wlow": np.ascontiguousarray(WlowA.transpose(1, 0, 2)).astype(bf16),
        "whigh": W_high.astype(bf16),
        "gmat": np.ascontiguousarray(Gr.transpose(1, 0, 2)).astype(bf16),
        "maskT": np.ascontiguousarray(maskTr.transpose(1, 0, 2)).astype(bf16),
        "wsrcrep": np.ascontiguousarray(
            wsrcrep.transpose(2, 0, 1, 3)).astype(bf16),
    }
    in_maps = []
    for b in range(B):
        xt = np.ascontiguousarray(x[b].T)                    # [H, N]
        m = dict(shared)
        m["xt"] = np.ascontiguousarray(
            xt.reshape(CCH, 128, N).transpose(1, 0, 2)).astype(bf16)
        m["xres"] = np.ascontiguousarray(
            x[b].reshape(NCH, 128, H).transpose(1, 0, 2))
        in_maps.append(m)
    return in_maps, trivial


def kernel(**inputs) -> np.ndarray:
    in_maps, trivial = _prep(inputs)
    key = "k"
    if key not in _CACHED:
        _CACHED[key] = _build(trivial)
    nc = _CACHED[key]
    res = run_bass_kernel_spmd(nc, in_maps, list(range(B)))
    out = np.stack([res.results[i]["out"] for i in range(B)], axis=0)
    return out.astype(np.float32)


if __name__ == "__main__":
    import reference
    inputs = {k: np.asarray(v) for k, v in reference.setup_inputs().items()}
    got = kernel(**inputs)
    print("kernel output", got.shape, got.dtype)
